# revision 58
# baseline (speedup 1.0000x reference)
"""Multi-head attention (B=4,N=2048,C=768,H=12) on 8 trn2 NeuronCores.

Sharding: data-parallel over B (4 batches x 2 cores each), tensor-parallel
over heads (6 heads per core). Each core (all-bf16 inputs, fp32 psum):
  - QKV projection emitted kt-OUTER in 8-group waves in a dedicated psum
    pool so PE tracks the xT DMA stream without head-of-line blocking
  - scores st[kv, q]: contraction D=64, two heads row-packed into PE
    partitions 0-63/64-127; st psum ring depth 3 hides the exp latency
  - exp: 10/16 kv tiles on ScalarE (scale fused); 6/16 via a Schraudolph
    fast-exp on DVE (int16(A*x*s + B) bit pattern = bf16 exp, ~2% rms) so
    neither engine paces the strip
  - attn@V flipped: est stationary ([128 kv, 128 q]), V+ones column (65)
    moving -> psum [q, 65]; softmax denominator lands per q PARTITION so
    normalization is one strided reciprocal + per-partition tensor_scalar.
    HW constraint: interleaved psum accumulation groups must sit in
    DISTINCT banks (same-bank interleave drops the first term), so only
    qsub0 of a strip accumulates during its own kt loop (2 bank-aligned
    slots); qsubs 1-3 of strip s are spread through strip s+1's kt loop
    (8 matmuls/iter) into the same two banks at disjoint column ranges,
    strictly sequentially per group. est tiles persist 34 deep for that.
  - normalized tiles are transposed to proj layout [hd, n] by XBAR
    dma_start_transpose (sbuf->sbuf) during attention, when DMA is idle
  - end phase: output projection with contraction 384 (3 pair-chunks in
    psum), bf16 y staged 4 ntiles per DMA to amortize HWDGE/SP overhead;
    the final strip's attnV retires through the first proj iterations
Host sums the two cores' partials per batch and adds the bias.
"""

import sys

import numpy as np
import ml_dtypes

_REPO = "/opt/trn_rl_repo"
if _REPO not in sys.path:
    sys.path.insert(0, _REPO)

import concourse.bacc as bacc
import concourse.mybir as mybir
import concourse.tile as tile
from concourse.bass_utils import run_bass_kernel_spmd

B, N, C, H, D = 4, 2048, 768, 12, 64
HL = H // 2          # heads per core
SCALE = D ** -0.5
NCORES = 8
KT_C = C // 128      # 6 contraction tiles over C
QS = N // 512        # 4 query strips
KVT = N // 128       # 16 kv tiles
PAIRS = HL // 2      # 3 head pairs per core

F32 = mybir.dt.float32
BF16 = mybir.dt.bfloat16
I16 = mybir.dt.int16
EXP = mybir.ActivationFunctionType.Exp
MULT = mybir.AluOpType.mult
ADD = mybir.AluOpType.add

# Schraudolph fast-exp (bf16 bit pattern): bits = round(x*SCALE*FA + FB)
FA = 184.6650390625          # 2^7 / ln 2
FB = 16256.0 - 5.5           # 127*2^7 minus balance offset
# kv tiles per strip whose exp runs on DVE instead of ACT
FAST_KTS = (1, 3, 6, 9, 11, 14)

_CACHE = {}


def _build():
    nc = bacc.Bacc("TRN2", target_bir_lowering=False, debug=False,
                   num_devices=NCORES)
    xT = nc.dram_tensor("xT", [C, N], BF16, kind="ExternalInput").ap()
    wqkvT = nc.dram_tensor("wqkvT", [C, 3 * HL * D], BF16, kind="ExternalInput").ap()
    wpT = nc.dram_tensor("wpT", [HL * D, C], BF16, kind="ExternalInput").ap()
    y = nc.dram_tensor("y", [N, C], BF16, kind="ExternalOutput").ap()

    with tile.TileContext(nc) as tc:
        with (
            tc.tile_pool(name="singles", bufs=1) as singles,
            tc.tile_pool(name="est", bufs=34) as est_p,
            tc.tile_pool(name="rec", bufs=2) as rec_p,
                        tc.tile_pool(name="ysb", bufs=3) as ysb_p,
        ):
            xT_sb = singles.tile([128, KT_C, N], BF16)
            wqkv_sb = singles.tile([128, KT_C, 3 * HL * D], BF16)
            wqk_sb = wqkv_sb[:, :, 0:2 * HL * D]
            wv_sb = wqkv_sb[:, :, 2 * HL * D:]
            wp_sb = singles.tile([128, PAIRS, C], BF16)
            qk_sb = singles.tile([128, 2 * PAIRS, N], BF16)
            v_sb = singles.tile([128, KVT, HL, D + 1], BF16)
            # normalized attention, pre-transpose: [q, pair, qtile, 2*64]
            attnN = singles.tile([128, PAIRS, KVT, 128], BF16)
            # XBAR-transposed proj operand: [hd-pair, pair, ntile, 128]
            trsT = singles.tile([128, PAIRS, KVT, 128], BF16)
            warm_sb = singles.tile([128, 640], BF16)

            # warm-up data first so the PE warm matmuls and the ACT exp
            # table load never wait behind the DMA stream
            nc.vector.memset(warm_sb, 0.0)
            warm_in = rec_p.tile([1, 2], F32, tag="warm")
            warm_out = rec_p.tile([1, 2], BF16, tag="warmo")
            nc.vector.memset(warm_in, 0.0)
            nc.scalar.activation(warm_out, warm_in, EXP, scale=SCALE)

            # DMA order kt-major so contraction step kt has all it needs
            for kt in range(KT_C):
                nc.sync.dma_start(xT_sb[:, kt, :], xT[kt * 128:(kt + 1) * 128, :])
                nc.sync.dma_start(wqkv_sb[:, kt, :],
                                  wqkvT[kt * 128:(kt + 1) * 128, :])
            for p in range(PAIRS):
                nc.sync.dma_start(wp_sb[:, p, :], wpT[p * 128:(p + 1) * 128, :])
            nc.vector.memset(v_sb[:, :, :, D:D + 1], 1.0)

            # ---- fill phase: 8-group kt-outer waves in a dedicated pool ----
            with tc.tile_pool(name="fill", bufs=1, space="PSUM") as fill_p:
                # PE warm-up (keeps the HAM clock-gate at 2.4GHz) while the
                # first xT tiles are in flight
                for i in range(8):
                    warm_ps = fill_p.tile([128, 512], F32, tag=f"f{i}",
                                          name="warmps")
                    nc.tensor.matmul(warm_ps, lhsT=warm_sb[:, 0:128],
                                     rhs=warm_sb[:, 128:640])

                def emit_wave(groups, kt_outer=True):
                    """groups: list of (kind, idx). kind 'qk': idx=(t, qs);
                    kind 'v': idx=mt. kt-outer while the xT DMA stream is
                    still landing; group-major once resident so each copy
                    hides behind the next group's matmuls."""
                    tiles = []
                    for i, _ in enumerate(groups):
                        ps = fill_p.tile([128, 512], F32, tag=f"f{i}",
                                         name="fillps")
                        tiles.append(ps)
                    order = ([(kt, g) for kt in range(KT_C)
                              for g in range(len(groups))] if kt_outer else
                             [(kt, g) for g in range(len(groups))
                              for kt in range(KT_C)])
                    for kt, g in order:
                        for kind, idx in [groups[g]]:
                            ps = tiles[g]
                            if kind == "qk":
                                t, qs = idx
                                nc.tensor.matmul(
                                    ps,
                                    lhsT=wqk_sb[:, kt, t * 128:(t + 1) * 128],
                                    rhs=xT_sb[:, kt, qs * 512:(qs + 1) * 512],
                                    start=(kt == 0), stop=(kt == KT_C - 1),
                                )
                            else:
                                mt = idx
                                nc.tensor.matmul(
                                    ps[:, 0:HL * D],
                                    lhsT=xT_sb[:, kt, mt * 128:(mt + 1) * 128],
                                    rhs=wv_sb[:, kt, :],
                                    start=(kt == 0), stop=(kt == KT_C - 1),
                                )
                            if kt != KT_C - 1:
                                continue
                            # copy out right after this group's stop,
                            # alternating engines so no single copy tail
                            # delays the bank reuse by the attention pools
                            if kind == "qk":
                                eng = nc.vector.tensor_copy if g % 2 else nc.scalar.copy
                                eng(qk_sb[:, t, qs * 512:(qs + 1) * 512], ps)
                            else:
                                out = v_sb[:, mt, :, 0:D]
                                src_ = ps[:, 0:HL * D].rearrange(
                                    "p (h d) -> p h d", h=HL)
                                if g % 2:
                                    nc.vector.tensor_copy(out, src_)
                                else:
                                    nc.scalar.copy(out, src_)

                emit_wave([("qk", (0, qs)) for qs in range(QS)]
                          + [("qk", (PAIRS, qs)) for qs in range(QS)])
                emit_wave([("v", mt) for mt in range(8)], kt_outer=False)
                emit_wave([("v", mt) for mt in range(8, 12)], kt_outer=False)

            ps_att_cm = tc.tile_pool(name="ps_att", bufs=1, space="PSUM")
            ps_att = ps_att_cm.__enter__()
            with tc.tile_pool(name="ps_st", bufs=3, space="PSUM") as ps_st:
                def emit_qk_tile(t):
                    for qs in range(QS):
                        ps = ps_st.tile([128, 512], F32, tag="st", name="qkps")
                        for kt in range(KT_C):
                            nc.tensor.matmul(
                                ps,
                                lhsT=wqk_sb[:, kt, t * 128:(t + 1) * 128],
                                rhs=xT_sb[:, kt, qs * 512:(qs + 1) * 512],
                                start=(kt == 0), stop=(kt == KT_C - 1),
                            )
                        eng = nc.scalar.copy if qs % 2 else nc.vector.tensor_copy
                        eng(qk_sb[:, t, qs * 512:(qs + 1) * 512], ps)

                def emit_pass_unit(sd, g):
                    """One attnV matmul of a finished strip: global unit
                    g in [0,128): qsub = g//32, kt = (g%32)//2, h = g%2.
                    Groups live at disjoint column ranges of the same two
                    pa banks; qsub groups are strictly sequential in PE
                    order, which the HW psum accumulator requires."""
                    qsub, kt, h = g // 32, (g % 32) // 2, g % 2
                    nc.tensor.matmul(
                        sd["pa"][:, h, qsub * 128:qsub * 128 + D + 1],
                        lhsT=sd["ests"][kt][:, h, qsub * 128:(qsub + 1) * 128],
                        rhs=v_sb[:, kt, 2 * sd["pr"] + h, :],
                        start=(kt == 0), stop=(kt == KVT - 1),
                    )

                def emit_pass_norm(sd, qsub, dma_eng=None):
                    pr, qs = sd["pr"], sd["qs"]
                    qt = qs * 4 + qsub
                    rec = rec_p.tile([128, 2], F32, tag="rec")
                    nc.vector.reciprocal(
                        rec,
                        sd["pa"][:, :, qsub * 128 + D:qsub * 128 + D + 1]
                        .rearrange("p a b -> p (a b)"))
                    for h in range(2):
                        nc.vector.tensor_scalar_mul(
                            attnN[:, pr, qt, h * D:(h + 1) * D],
                            sd["pa"][:, h, qsub * 128:qsub * 128 + D],
                            rec[:, h:h + 1])
                    (dma_eng or nc.sync).dma_start_transpose(
                        trsT[:, pr, qt, :], attnN[:, pr, qt, :])

                def drain_strip(sd, lo, hi, dma_eng=None):
                    """Emit pass units [lo,hi) and any due normalizes."""
                    if sd is None:
                        return
                    for g in range(lo, hi):
                        emit_pass_unit(sd, g)
                        if (g + 1) % 32 == 0:
                            emit_pass_norm(sd, g // 32, dma_eng)

                def emit_attention_strip(pr, qs, prev, vload=None):
                    """Scores+exp for (pr, qs); attnV of the PREVIOUS strip
                    is spread through this kt loop (8 units/iter) so the PE
                    stream has no burst and ACT/DVE never idle. vload (first
                    strip only, which has no drain) streams leftover V
                    projection groups through the idle pa banks instead."""
                    tq, tk = pr, PAIRS + pr
                    qsl = slice(qs * 512, (qs + 1) * 512)
                    ests = []
                    if prev is not None:
                        prev["pa"] = ps_att.tile([128, 2, 512], F32, tag="pa",
                                                 name="pa")
                    vp = None
                    if vload is not None:
                        vp = ps_att.tile([128, 2, 512], F32, tag="pa",
                                         name="vp")

                    def emit_vwork(it):
                        # rounds of 2 concurrent groups (distinct banks),
                        # kt-steps then a DVE copy-out per round
                        rnd, ph = divmod(it, 8)
                        if rnd >= len(vload) // 2:
                            return
                        r0 = 2 * rnd
                        if ph >= KT_C:
                            if ph == KT_C:
                                for g in range(2):
                                    mt = vload[r0 + g]
                                    nc.vector.tensor_copy(
                                        v_sb[:, mt, :, 0:D],
                                        vp[:, g, 0:HL * D].rearrange(
                                            "p (h d) -> p h d", h=HL))
                            return
                        for g in range(2):
                            nc.tensor.matmul(
                                vp[:, g, 0:HL * D],
                                lhsT=xT_sb[:, ph, (vload[r0 + g]) * 128:
                                           (vload[r0 + g] + 1) * 128],
                                rhs=wv_sb[:, ph, :],
                                start=(ph == 0), stop=(ph == KT_C - 1),
                            )

                    for kt in range(KVT):
                        st = ps_st.tile([128, 2, 512], F32, tag="st")
                        for half in range(2):
                            p0, p1 = half * 64, (half + 1) * 64
                            nc.tensor.matmul(
                                st[:, half, :],
                                lhsT=qk_sb[p0:p1, tk, kt * 128:(kt + 1) * 128],
                                rhs=qk_sb[p0:p1, tq, qsl],
                            )
                        est = est_p.tile([128, 2, 512], BF16, tag="est")
                        if kt in FAST_KTS:
                            nc.vector.tensor_scalar(
                                est.bitcast(I16), st, SCALE * FA, FB,
                                op0=MULT, op1=ADD)
                        else:
                            nc.scalar.activation(est, st, EXP, scale=SCALE)
                        ests.append(est)
                        drain_strip(prev, kt * 8, (kt + 1) * 8)
                        if vload is not None:
                            emit_vwork(kt)
                    return {"pr": pr, "qs": qs, "ests": ests}

                prev = None
                for pr in range(PAIRS):
                    for qs in range(QS):
                        vload = [12, 13, 14, 15] if prev is None else None
                        prev = emit_attention_strip(pr, qs, prev, vload)
                    if pr + 1 < PAIRS:
                        emit_qk_tile(pr + 1)
                        emit_qk_tile(PAIRS + pr + 1)
                final = prev

                # final strip's pa lives in ps_att, which stays open into
                # the end phase so the spread-out flush never collides with
                # ps_y's reuse of the st banks
                final["pa"] = ps_att.tile([128, 2, 512], F32, tag="pa",
                                          name="pa")

            # end phase: project straight from the XBAR-transposed tiles
            with tc.tile_pool(name="ps_y", bufs=6, space="PSUM") as ps_y:
                ysb = None
                # y quads: 4+4+4+2+2 so the tail DMAs issue early
                ybatch = {0: 4, 4: 4, 8: 4, 12: 3, 15: 1}
                ystart = 0
                for mt in range(KVT):
                    drain_strip(final, min(mt * 16, 128),
                                min((mt + 1) * 16, 128))
                    if mt in ybatch:
                        ystart, yn = mt, ybatch[mt]
                        ysb = ysb_p.tile([128, yn, 2, 384], BF16, tag="ysb",
                                         name="ysb")
                    for ns in range(2):
                        yp = ps_y.tile([128, 384], F32, tag="yp")
                        for pr in range(PAIRS):
                            nc.tensor.matmul(
                                yp,
                                lhsT=trsT[:, pr, mt, :],
                                rhs=wp_sb[:, pr, ns * 384:(ns + 1) * 384],
                                start=(pr == 0), stop=(pr == PAIRS - 1),
                            )
                        if ns == 0:
                            nc.scalar.copy(ysb[:, mt - ystart, ns, :], yp)
                        else:
                            nc.vector.tensor_copy(ysb[:, mt - ystart, ns, :], yp)
                    if mt - ystart == yn - 1:
                        nc.sync.dma_start(
                            y[ystart * 128:(mt + 1) * 128, :]
                            .rearrange("(a p) c -> p a c", p=128),
                            ysb.rearrange("p a b c -> p a (b c)"))

            ps_att_cm.__exit__(None, None, None)

    nc.compile()
    return nc


def _get_nc():
    if "nc" not in _CACHE:
        _CACHE["nc"] = _build()
    return _CACHE["nc"]


def _prep_inputs(x, w_qkv, w_proj):
    """Per-core input dicts. Core c: batch c//2, head-half c%2."""
    wq, wk, wv = w_qkv[0:C], w_qkv[C:2 * C], w_qkv[2 * C:3 * C]
    in_maps = []
    for core in range(NCORES):
        b, p = divmod(core, 2)
        heads = [p * HL + j for j in range(HL)]
        qk_rows = np.concatenate(
            [wq[h * D:(h + 1) * D] for h in heads]
            + [wk[h * D:(h + 1) * D] for h in heads], axis=0)   # [768, C]
        v_rows = np.concatenate(
            [wv[h * D:(h + 1) * D] for h in heads], axis=0)     # [384, C]
        p_cols = np.concatenate(
            [w_proj[:, h * D:(h + 1) * D] for h in heads], axis=1)  # [C, 384]
        qkv_rows = np.concatenate([qk_rows, v_rows], axis=0)  # [1152, C]
        in_maps.append({
            "xT": np.ascontiguousarray(x[b].T).astype(ml_dtypes.bfloat16),
            "wqkvT": np.ascontiguousarray(qkv_rows.T).astype(ml_dtypes.bfloat16),
            "wpT": np.ascontiguousarray(p_cols.T).astype(ml_dtypes.bfloat16),
        })
    return in_maps


def kernel(x, w_qkv, w_proj, b_proj, _trace=False):
    x = np.asarray(x, dtype=np.float32)
    w_qkv = np.asarray(w_qkv, dtype=np.float32)
    w_proj = np.asarray(w_proj, dtype=np.float32)
    b_proj = np.asarray(b_proj, dtype=np.float32)

    nc = _get_nc()
    in_maps = _prep_inputs(x, w_qkv, w_proj)
    # retry: transient NRT_EXEC_UNIT_UNRECOVERABLE has been observed once
    # on a first attempt and succeeded immediately on retry
    last_exc = None
    for _attempt in range(3):
        try:
            res = run_bass_kernel_spmd(nc, in_maps,
                                       core_ids=list(range(NCORES)),
                                       trace=_trace)
            break
        except Exception as e:
            last_exc = e
    else:
        raise last_exc
    _CACHE["last_results"] = res

    out = np.empty((B, N, C), dtype=np.float32)
    for b in range(B):
        out[b] = (res.results[2 * b]["y"].astype(np.float32)
                  + res.results[2 * b + 1]["y"].astype(np.float32) + b_proj)
    return out
